# revision 17
# baseline (speedup 1.0000x reference)
"""Biaffine kernel for Trainium2, data-parallel over batch across 8 NeuronCores.

Reference math (per batch b):
    Daug = [D, 1]                                  # [S, d+1]
    out  = Daug @ U @ H^T + (Daug @ W[:d+1])[:, None] + (H @ W[d+1:])[None, :]

Algebraic refactor used here (d = 1024):
    U0 = U[:d]                # [d, d]
    c  = U[d] + W[d+1:]       # [d]  (folds the ones-row of Daug and the H linear term)
    T' = D @ U0 + c           # [S, d]
    dlin = D @ W[:d] + W[d]   # [S]  (tiny; computed host-side)
    out  = T' @ H^T + dlin[:, None]

Device kernel per core (4 batches, 384 matmuls):
    matmul1: T'^T[j, x] = sum_k U0[k, j] * D^T[k, x]  (lhsT = U0, rhs = D^T)
             + per-partition bias c fused into the PSUM->SBUF copy
    matmul2: out[x, y] = sum_j T'^T[j, x] * H^T[j, y] (lhsT = T'^T, rhs = H^T)
             + per-partition bias dlin fused into the PSUM->SBUF copy

Performance structure:
  - Matmul operands are float16 (1 cycle/row on the PE like fp32r, but HALF the
    DMA bytes: 10 MB of input loads per core instead of 20 MB). fp16's 10-bit
    mantissa keeps end-to-end relative error ~4e-4, entirely from input
    quantization; PSUM accumulation is exact fp32.
  - D^T / H^T / U0 are transposed AND pre-swizzled host-side to the exact SBUF
    layout, so the device does zero transposes and every DMA partition read is
    one contiguous block.
  - Each DMA_DIRECT2D issue op costs ~600ns on the issuing engine, so loads are
    BATCHED: fine per-kt chunks only for the first few (which gate pipeline
    start), coarse 0.5-2MB transfers for everything else, split across both
    HWDGE rings (sync: U0 + later-batch D^T + even stores; scalar: batch-0
    D^T/H^T + later H^T + odd stores) in consumption order.
  - Batch 0's matmul1 runs kt-outer in two jm-half groups so the PE consumes
    chunks at DMA arrival rate AND the first half's PSUM->SBUF copies overlap
    the second half's matmuls.
  - PSUM->SBUF bias copies alternate between the DVE (tensor_scalar_add) and
    the Activation engine (Identity w/ AP bias) so copy supply (~175ns/tile
    effective) keeps up with matmul2's lhsT consumption (216ns/tile).
  - A few short fp16 warm-up matmuls on a memset tile occupy the PE during the
    startup DMA window so the HAM clock-gate ramp starts immediately.
  - The final output tile's matmul2 is split into 4 column groups so its bias
    copies and stores (alternating rings) pipeline with the last matmuls,
    shrinking the drain tail.

BIAFFINE_MM_F32R=1 switches back to fp32r matmuls (rel err ~2e-4, slower DMA).
"""
import os
import sys

import numpy as np

for _p in (
    "/root/.axon_site",
    "/root/.axon_site/_ro/trn_rl_repo",
    "/root/.axon_site/_ro/pypackages",
    "/opt/trn_rl_repo",
):
    if os.path.isdir(_p) and _p not in sys.path:
        sys.path.append(_p)

import concourse.bass as bass
import concourse.mybir as mybir
import concourse.tile as tile
from concourse import bacc
from concourse.bass_utils import run_bass_kernel_spmd

B, S, D_DIM = 32, 512, 1024
N_CORES = 8
BPC = B // N_CORES  # batches per core
KT = D_DIM // 128  # 8 k-tiles (contraction over d)
JT = D_DIM // 128  # 8 j-tiles (M dim of matmul1)
XT = S // 128  # 4 x-tiles (M dim of matmul2)

_NC_CACHE = {}


def _use_f16() -> bool:
    return os.environ.get("BIAFFINE_MM_F32R", "0") != "1"


def _build_nc() -> bass.Bass:
    nc = bacc.Bacc()
    f32 = mybir.dt.float32
    mm_dt = mybir.dt.float16 if _use_f16() else mybir.dt.float32r
    ident = mybir.ActivationFunctionType.Identity

    # Inputs arrive pre-swizzled to the SBUF layout: [.., p, kt, x] so each
    # partition's DMA read is one contiguous block.
    dt_in = nc.dram_tensor("dt_in", [BPC, 128, KT, S], mm_dt, kind="ExternalInput")
    ht_in = nc.dram_tensor("ht_in", [BPC, 128, KT, S], mm_dt, kind="ExternalInput")
    u0_in = nc.dram_tensor("u0_in", [128, KT, D_DIM], mm_dt, kind="ExternalInput")
    ccol_in = nc.dram_tensor("ccol_in", [128, JT], f32, kind="ExternalInput")
    dcol_in = nc.dram_tensor("dcol_in", [128, BPC * XT], f32, kind="ExternalInput")
    out_t = nc.dram_tensor("out", [BPC, S, S], f32, kind="ExternalOutput")

    with tile.TileContext(nc) as tc:
        with (
            tc.tile_pool(name="const", bufs=1) as cpool,
            tc.tile_pool(name="dh", bufs=2) as dh_pool,
            tc.tile_pool(name="tt", bufs=2) as tt_pool,
            tc.tile_pool(name="ot", bufs=3) as ot_pool,
            tc.tile_pool(name="ps", bufs=8, space="PSUM") as ps_pool,
        ):
            # HAM warm-up: short matmuls on a memset tile fill the startup DMA
            # window so the PE p-state ramp starts as early as possible.
            warm_sb = cpool.tile([128, 128], mm_dt, name="warm_sb")
            nc.vector.memset(warm_sb[:], 0.0)
            # ~40 short matmuls bridge the first-chunk DMA latency (~4us:
            # issue + queue pickup + transfer) so the PE p-state ramp runs
            # uninterrupted and data matmuls start at the full 2.4GHz clock.
            warm_ps = ps_pool.tile([128, S], f32, tag="ps", name="warm_ps")
            for _ in range(48):
                nc.tensor.matmul(
                    warm_ps[:, :128], lhsT=warm_sb[:], rhs=warm_sb[:], start=True,
                    stop=True,
                )

            # Batch-0 loads: fine chunks first (they gate pipeline start),
            # then coarse batches; u0/D^T pairs land in consumption order.
            # kt -> (tile, column offset) maps for the uneven chunking.
            # Chunks keep per-partition DMA elements >= 2KB: 1KB/512B-element
            # transfers run at a fraction of ring bandwidth. Interleave on
            # each queue in consumption order (each queued DMA adds ~0.8us of
            # pickup/hole latency, so order matters more than balance).
            u0_sizes = [1, 1, 2, 2, 2]  # 5 issues on sync
            dt0_sizes = [2, 2, 2, 2]  # first chunk on sync, rest on scalar
            ht0_sizes = [2, 2, 2, 2]  # 4 issues on scalar

            def alloc_chunks(sizes, width, make_tile):
                chunks, index = [], []
                k0 = 0
                for n in sizes:
                    t = make_tile(n, k0)
                    chunks.append((t, k0, n))
                    index += [(t, i * width) for i in range(n)]
                    k0 += n
                return chunks, index

            u0_c, u0_x = alloc_chunks(
                u0_sizes,
                D_DIM,
                lambda n, k0: cpool.tile([128, n * D_DIM], mm_dt, name=f"u0k{k0}"),
            )
            dt_c, dt_x = alloc_chunks(
                dt0_sizes,
                S,
                lambda n, k0: dh_pool.tile(
                    [128, n * S], mm_dt, tag=f"dt{k0}", name=f"dt{k0}"
                ),
            )
            ht_c, ht_x = alloc_chunks(
                ht0_sizes,
                S,
                lambda n, k0: dh_pool.tile(
                    [128, n * S], mm_dt, tag=f"ht{k0}", name=f"ht{k0}"
                ),
            )
            ccol = cpool.tile([128, JT], f32)
            dcol = cpool.tile([128, BPC * XT], f32)

            def issue(eng, chunk, src):
                t, k0, n = chunk
                eng.dma_start(t[:], src[:, k0 : k0 + n, :])

            # Per-queue issue order = consumption order. sync: the u0 stream
            # plus the very first dt chunk (so kt0 doesn't wait on the slower
            # scalar-queue cold start); scalar: remaining dt chunks, then ht.
            # Tiny first transfers absorb each queue's cold-start pickup
            # latency before the critical first chunks.
            nc.sync.dma_start(ccol[:], ccol_in[:])
            nc.scalar.dma_start(dcol[:], dcol_in[:])
            issue(nc.sync, u0_c[0], u0_in)
            issue(nc.scalar, dt_c[0], dt_in[0])
            issue(nc.sync, u0_c[1], u0_in)
            issue(nc.scalar, dt_c[1], dt_in[0])
            issue(nc.sync, u0_c[2], u0_in)
            issue(nc.scalar, dt_c[2], dt_in[0])
            issue(nc.sync, u0_c[3], u0_in)
            issue(nc.scalar, dt_c[3], dt_in[0])
            for hc in ht_c:
                issue(nc.scalar, hc, ht_in[0])
            issue(nc.sync, u0_c[4], u0_in)

            def u0_ap(kt, jm):
                t, off = u0_x[kt]
                return t[:, off + jm * 128 : off + (jm + 1) * 128]

            def chunk_ap(x, kt, cols=None):
                t, off = x[kt]
                if cols is None:
                    return t[:, off : off + S]
                return t[:, off + cols.start : off + cols.stop]

            def tt_copy(jm, tt_tile, ps_tile):
                # PSUM->SBUF copy with fused per-partition bias c; alternate
                # engines so two copies proceed in parallel.
                if jm % 2 == 0:
                    nc.vector.tensor_scalar_add(
                        tt_tile[:], ps_tile[:], ccol[:, jm : jm + 1]
                    )
                else:
                    nc.scalar.activation(
                        tt_tile[:], ps_tile[:], ident, bias=ccol[:, jm : jm + 1]
                    )

            for b in range(BPC):
                if b > 0:
                    # Steady-state loads: one coarse DMA per tensor (issue-op
                    # economy); they land long before they are consumed.
                    dt_b = dh_pool.tile([128, KT * S], mm_dt, tag="dtb", name="dtb")
                    nc.sync.dma_start(dt_b[:], dt_in[b])
                    ht_b = dh_pool.tile([128, KT * S], mm_dt, tag="htb", name="htb")
                    nc.scalar.dma_start(ht_b[:], ht_in[b])
                    dt_x = [(dt_b, kt * S) for kt in range(KT)]
                    ht_x = [(ht_b, kt * S) for kt in range(KT)]

                # matmul1: T'^T[jm*128+p, x]  (+ bias c)
                tt_t = [
                    tt_pool.tile([128, S], mm_dt, tag=f"tt{jm}", name=f"tt{jm}") for jm in range(JT)
                ]
                if b == 0:
                    # kt-outer across all 8 jm: each kt step needs only chunk
                    # kt (1.73us of matmuls per chunk tracks DMA arrivals);
                    # the per-jm copies stagger off the last kt round fast
                    # enough (2 engines) that matmul2 never stalls.
                    ps_l = [
                        ps_pool.tile([128, S], f32, tag="ps", name=f"ps{jm}")
                        for jm in range(JT)
                    ]
                    for kt in range(KT):
                        for jm in range(JT):
                            nc.tensor.matmul(
                                ps_l[jm][:],
                                lhsT=u0_ap(kt, jm),
                                rhs=chunk_ap(dt_x, kt),
                                start=(kt == 0),
                                stop=(kt == KT - 1),
                            )
                    for jm in range(JT):
                        tt_copy(jm, tt_t[jm], ps_l[jm])
                else:
                    for jm in range(JT):
                        ps = ps_pool.tile([128, S], f32, tag="ps", name="ps")
                        for kt in range(KT):
                            nc.tensor.matmul(
                                ps[:],
                                lhsT=u0_ap(kt, jm),
                                rhs=chunk_ap(dt_x, kt),
                                start=(kt == 0),
                                stop=(kt == KT - 1),
                            )
                        tt_copy(jm, tt_t[jm], ps)

                # matmul2: out[xt*128+p, y]  (+ bias dlin)
                for xt in range(XT):
                    dc = dcol[:, b * XT + xt : b * XT + xt + 1]
                    if b == BPC - 1 and xt == XT - 1:
                        # Final tile: 4 column groups in SEPARATE PSUM tiles
                        # (a shared tile would serialize group g+1's matmuls
                        # behind group g's copy) so bias copies + stores
                        # pipeline with the last matmuls. The slow scalar
                        # queue gets only the earliest strip.
                        ot = ot_pool.tile([128, S], f32, tag="ot", name="ot")
                        # Uneven strips: the LAST one is smallest since its
                        # copy+store latency is the drain's critical path.
                        bounds = [0, 160, 320, 448, 512]
                        for g in range(4):
                            cs = slice(bounds[g], bounds[g + 1])
                            w = bounds[g + 1] - bounds[g]
                            po = ps_pool.tile([128, S], f32, tag="ps", name=f"po{g}")
                            for jm in range(JT):
                                nc.tensor.matmul(
                                    po[:, :w],
                                    lhsT=tt_t[jm][:, xt * 128 : (xt + 1) * 128],
                                    rhs=chunk_ap(ht_x, jm, cs),
                                    start=(jm == 0),
                                    stop=(jm == JT - 1),
                                )
                            # Copies on the DVE only: the scalar ENGINE must
                            # stay free to issue its strip stores promptly.
                            nc.vector.tensor_scalar_add(ot[:, cs], po[:, :w], dc)
                            (nc.scalar if g % 2 == 0 else nc.sync).dma_start(
                                out_t[b, xt * 128 : (xt + 1) * 128, cs], ot[:, cs]
                            )
                    else:
                        po = ps_pool.tile([128, S], f32, tag="ps", name="po")
                        for jm in range(JT):
                            nc.tensor.matmul(
                                po[:],
                                lhsT=tt_t[jm][:, xt * 128 : (xt + 1) * 128],
                                rhs=chunk_ap(ht_x, jm),
                                start=(jm == 0),
                                stop=(jm == JT - 1),
                            )
                        ot = ot_pool.tile([128, S], f32, tag="ot", name="ot")
                        nc.vector.tensor_scalar_add(ot[:], po[:], dc)
                        # b3-xt2 goes on scalar so the sync queue is empty
                        # for the final tile's strips right behind it.
                        ring = nc.sync if xt % 2 == 0 else nc.scalar
                        if b == BPC - 1 and xt == XT - 2:
                            ring = nc.scalar
                        ring.dma_start(
                            out_t[b, xt * 128 : (xt + 1) * 128, :], ot[:]
                        )
    nc.finalize()
    return nc


def _get_nc() -> bass.Bass:
    key = "nc_f16" if _use_f16() else "nc_f32r"
    if key not in _NC_CACHE:
        _NC_CACHE[key] = _build_nc()
    return _NC_CACHE[key]


def _round_fp32r(a: np.ndarray) -> np.ndarray:
    """Round fp32 to fp32r layout: RNE to 11-bit mantissa, low 12 bits zero."""
    bits = np.ascontiguousarray(a, dtype=np.float32).view(np.uint32)
    odd = (bits >> 12) & np.uint32(1)
    out = (bits + np.uint32(0x7FF) + odd) & np.uint32(0xFFFFF000)
    return out.view(np.float32)


def kernel(D, H, U, W):
    D = np.ascontiguousarray(np.asarray(D, dtype=np.float32))
    H = np.ascontiguousarray(np.asarray(H, dtype=np.float32))
    U = np.asarray(U, dtype=np.float32)
    W = np.asarray(W, dtype=np.float32)
    d = D_DIM
    f16 = _use_f16()

    def _mm_cast(a):
        return a.astype(np.float16) if f16 else _round_fp32r(a)

    # U0 swizzled to [128, KT, d]: [p, kt, j] = U0[kt*128+p, j]
    U0 = _mm_cast(
        np.ascontiguousarray(U[:d, :].reshape(KT, 128, d).transpose(1, 0, 2))
    )
    c = (U[d, :] + W[d + 1 :]).astype(np.float32)  # [d]
    # ccol[p, jm] = c[jm*128 + p]
    ccol = np.ascontiguousarray(c.reshape(JT, 128).T)
    # dlin[b, x] = D[b, x] . W[:d] + W[d]
    dlin = (D @ W[:d] + W[d]).astype(np.float32)  # [B, S]

    in_maps = []
    for cidx in range(N_CORES):
        sl = slice(cidx * BPC, (cidx + 1) * BPC)
        # [b, p, kt, x] = X[b, x, kt*128+p]  (transpose + swizzle in one copy)
        Dt = _mm_cast(D[sl].reshape(BPC, S, KT, 128).transpose(0, 3, 2, 1))
        Ht = _mm_cast(H[sl].reshape(BPC, S, KT, 128).transpose(0, 3, 2, 1))
        # dcol[p, b*XT + xt] = dlin[b, xt*128 + p]
        dcol = np.ascontiguousarray(
            dlin[sl].reshape(BPC, XT, 128).transpose(2, 0, 1).reshape(128, BPC * XT)
        )
        in_maps.append(
            {
                "dt_in": Dt,
                "ht_in": Ht,
                "u0_in": U0,
                "ccol_in": ccol,
                "dcol_in": dcol,
            }
        )

    nc = _get_nc()
    trace = bool(int(os.environ.get("BIAFFINE_TRACE", "0")))
    kwargs = {}
    if trace:
        tdir = os.environ.get("BIAFFINE_TRACE_DIR")
        if tdir:
            os.makedirs(tdir, exist_ok=True)
            kwargs["tmpdir"] = tdir
    res = run_bass_kernel_spmd(
        nc, in_maps, core_ids=list(range(N_CORES)), trace=trace, **kwargs
    )
    if trace and res.exec_time_ns is not None:
        print(f"HW exec time: {res.exec_time_ns} ns")

    out = np.concatenate([res.results[i]["out"] for i in range(N_CORES)], axis=0)
    return out


# revision 18
# speedup vs baseline: 1.0128x; 1.0128x over previous
"""Biaffine kernel for Trainium2, data-parallel over batch across 8 NeuronCores.

Reference math (per batch b):
    Daug = [D, 1]                                  # [S, d+1]
    out  = Daug @ U @ H^T + (Daug @ W[:d+1])[:, None] + (H @ W[d+1:])[None, :]

Algebraic refactor used here (d = 1024):
    U0 = U[:d]                # [d, d]
    c  = U[d] + W[d+1:]       # [d]  (folds the ones-row of Daug and the H linear term)
    T' = D @ U0 + c           # [S, d]
    dlin = D @ W[:d] + W[d]   # [S]  (tiny; computed host-side)
    out  = T' @ H^T + dlin[:, None]

Device kernel per core (4 batches, 384 matmuls):
    matmul1: T'^T[j, x] = sum_k U0[k, j] * D^T[k, x]  (lhsT = U0, rhs = D^T)
             + per-partition bias c fused into the PSUM->SBUF copy
    matmul2: out[x, y] = sum_j T'^T[j, x] * H^T[j, y] (lhsT = T'^T, rhs = H^T)
             + per-partition bias dlin fused into the PSUM->SBUF copy

Performance structure:
  - Matmul operands are float16 (1 cycle/row on the PE like fp32r, but HALF the
    DMA bytes: 10 MB of input loads per core instead of 20 MB). fp16's 10-bit
    mantissa keeps end-to-end relative error ~4e-4, entirely from input
    quantization; PSUM accumulation is exact fp32.
  - D^T / H^T / U0 are transposed AND pre-swizzled host-side to the exact SBUF
    layout, so the device does zero transposes and every DMA partition read is
    one contiguous block.
  - Each DMA_DIRECT2D issue op costs ~600ns on the issuing engine, so loads are
    BATCHED: fine per-kt chunks only for the first few (which gate pipeline
    start), coarse 0.5-2MB transfers for everything else, split across both
    HWDGE rings (sync: U0 + later-batch D^T + even stores; scalar: batch-0
    D^T/H^T + later H^T + odd stores) in consumption order.
  - Batch 0's matmul1 runs kt-outer in two jm-half groups so the PE consumes
    chunks at DMA arrival rate AND the first half's PSUM->SBUF copies overlap
    the second half's matmuls.
  - PSUM->SBUF bias copies alternate between the DVE (tensor_scalar_add) and
    the Activation engine (Identity w/ AP bias) so copy supply (~175ns/tile
    effective) keeps up with matmul2's lhsT consumption (216ns/tile).
  - A few short fp16 warm-up matmuls on a memset tile occupy the PE during the
    startup DMA window so the HAM clock-gate ramp starts immediately.
  - The final output tile's matmul2 is split into 4 column groups so its bias
    copies and stores (alternating rings) pipeline with the last matmuls,
    shrinking the drain tail.

BIAFFINE_MM_F32R=1 switches back to fp32r matmuls (rel err ~2e-4, slower DMA).
"""
import os
import sys

import numpy as np

for _p in (
    "/root/.axon_site",
    "/root/.axon_site/_ro/trn_rl_repo",
    "/root/.axon_site/_ro/pypackages",
    "/opt/trn_rl_repo",
):
    if os.path.isdir(_p) and _p not in sys.path:
        sys.path.append(_p)

import concourse.bass as bass
import concourse.mybir as mybir
import concourse.tile as tile
from concourse import bacc
from concourse.bass_utils import run_bass_kernel_spmd

B, S, D_DIM = 32, 512, 1024
N_CORES = 8
BPC = B // N_CORES  # batches per core
KT = D_DIM // 128  # 8 k-tiles (contraction over d)
JT = D_DIM // 128  # 8 j-tiles (M dim of matmul1)
XT = S // 128  # 4 x-tiles (M dim of matmul2)

_NC_CACHE = {}


def _use_f16() -> bool:
    return os.environ.get("BIAFFINE_MM_F32R", "0") != "1"


def _build_nc() -> bass.Bass:
    nc = bacc.Bacc()
    f32 = mybir.dt.float32
    mm_dt = mybir.dt.float16 if _use_f16() else mybir.dt.float32r
    ident = mybir.ActivationFunctionType.Identity

    # Inputs arrive pre-swizzled to the SBUF layout: [.., p, kt, x] so each
    # partition's DMA read is one contiguous block.
    dt_in = nc.dram_tensor("dt_in", [BPC, 128, KT, S], mm_dt, kind="ExternalInput")
    ht_in = nc.dram_tensor("ht_in", [BPC, 128, KT, S], mm_dt, kind="ExternalInput")
    u0_in = nc.dram_tensor("u0_in", [128, KT, D_DIM], mm_dt, kind="ExternalInput")
    ccol_in = nc.dram_tensor("ccol_in", [128, JT], f32, kind="ExternalInput")
    dcol_in = nc.dram_tensor("dcol_in", [128, BPC * XT], f32, kind="ExternalInput")
    out_t = nc.dram_tensor("out", [BPC, S, S], f32, kind="ExternalOutput")

    with tile.TileContext(nc) as tc:
        with (
            tc.tile_pool(name="const", bufs=1) as cpool,
            tc.tile_pool(name="dh", bufs=2) as dh_pool,
            tc.tile_pool(name="tt", bufs=2) as tt_pool,
            tc.tile_pool(name="ot", bufs=3) as ot_pool,
            tc.tile_pool(name="ps", bufs=8, space="PSUM") as ps_pool,
        ):
            # HAM warm-up: short matmuls on a memset tile fill the startup DMA
            # window so the PE p-state ramp starts as early as possible.
            warm_sb = cpool.tile([128, 128], mm_dt, name="warm_sb")
            nc.vector.memset(warm_sb[:], 0.0)
            # ~40 short matmuls bridge the first-chunk DMA latency (~4us:
            # issue + queue pickup + transfer) so the PE p-state ramp runs
            # uninterrupted and data matmuls start at the full 2.4GHz clock.
            warm_ps = ps_pool.tile([128, S], f32, tag="ps", name="warm_ps")
            for _ in range(48):
                nc.tensor.matmul(
                    warm_ps[:, :128], lhsT=warm_sb[:], rhs=warm_sb[:], start=True,
                    stop=True,
                )

            # Batch-0 loads: fine chunks first (they gate pipeline start),
            # then coarse batches; u0/D^T pairs land in consumption order.
            # kt -> (tile, column offset) maps for the uneven chunking.
            # Chunks keep per-partition DMA elements >= 2KB: 1KB/512B-element
            # transfers run at a fraction of ring bandwidth. Interleave on
            # each queue in consumption order (each queued DMA adds ~0.8us of
            # pickup/hole latency, so order matters more than balance).
            u0_sizes = [1, 1, 2, 2, 2]  # 5 issues on sync
            dt0_sizes = [2, 2, 2, 2]  # first chunk on sync, rest on scalar
            ht0_sizes = [2, 2, 2, 2]  # 4 issues on scalar

            def alloc_chunks(sizes, width, make_tile):
                chunks, index = [], []
                k0 = 0
                for n in sizes:
                    t = make_tile(n, k0)
                    chunks.append((t, k0, n))
                    index += [(t, i * width) for i in range(n)]
                    k0 += n
                return chunks, index

            u0_c, u0_x = alloc_chunks(
                u0_sizes,
                D_DIM,
                lambda n, k0: cpool.tile([128, n * D_DIM], mm_dt, name=f"u0k{k0}"),
            )
            dt_c, dt_x = alloc_chunks(
                dt0_sizes,
                S,
                lambda n, k0: dh_pool.tile(
                    [128, n * S], mm_dt, tag=f"dt{k0}", name=f"dt{k0}"
                ),
            )
            ht_c, ht_x = alloc_chunks(
                ht0_sizes,
                S,
                lambda n, k0: dh_pool.tile(
                    [128, n * S], mm_dt, tag=f"ht{k0}", name=f"ht{k0}"
                ),
            )
            ccol = cpool.tile([128, JT], f32)
            dcol = cpool.tile([128, BPC * XT], f32)

            def issue(eng, chunk, src):
                t, k0, n = chunk
                eng.dma_start(t[:], src[:, k0 : k0 + n, :])

            # Per-queue issue order = consumption order. sync: the u0 stream
            # plus the very first dt chunk (so kt0 doesn't wait on the slower
            # scalar-queue cold start); scalar: remaining dt chunks, then ht.
            issue(nc.sync, u0_c[0], u0_in)
            issue(nc.scalar, dt_c[0], dt_in[0])
            issue(nc.sync, u0_c[1], u0_in)
            issue(nc.scalar, dt_c[1], dt_in[0])
            issue(nc.sync, u0_c[2], u0_in)
            issue(nc.scalar, dt_c[2], dt_in[0])
            issue(nc.sync, u0_c[3], u0_in)
            issue(nc.scalar, dt_c[3], dt_in[0])
            for hc in ht_c:
                issue(nc.scalar, hc, ht_in[0])
            issue(nc.sync, u0_c[4], u0_in)
            nc.sync.dma_start(ccol[:], ccol_in[:])
            nc.scalar.dma_start(dcol[:], dcol_in[:])

            def u0_ap(kt, jm):
                t, off = u0_x[kt]
                return t[:, off + jm * 128 : off + (jm + 1) * 128]

            def chunk_ap(x, kt, cols=None):
                t, off = x[kt]
                if cols is None:
                    return t[:, off : off + S]
                return t[:, off + cols.start : off + cols.stop]

            def tt_copy(jm, tt_tile, ps_tile):
                # PSUM->SBUF copy with fused per-partition bias c; alternate
                # engines so two copies proceed in parallel.
                if jm % 2 == 0:
                    nc.vector.tensor_scalar_add(
                        tt_tile[:], ps_tile[:], ccol[:, jm : jm + 1]
                    )
                else:
                    nc.scalar.activation(
                        tt_tile[:], ps_tile[:], ident, bias=ccol[:, jm : jm + 1]
                    )

            for b in range(BPC):
                if b > 0:
                    # Steady-state loads: one coarse DMA per tensor (issue-op
                    # economy); they land long before they are consumed.
                    dt_b = dh_pool.tile([128, KT * S], mm_dt, tag="dtb", name="dtb")
                    nc.sync.dma_start(dt_b[:], dt_in[b])
                    ht_b = dh_pool.tile([128, KT * S], mm_dt, tag="htb", name="htb")
                    nc.scalar.dma_start(ht_b[:], ht_in[b])
                    dt_x = [(dt_b, kt * S) for kt in range(KT)]
                    ht_x = [(ht_b, kt * S) for kt in range(KT)]

                # matmul1: T'^T[jm*128+p, x]  (+ bias c)
                tt_t = [
                    tt_pool.tile([128, S], mm_dt, tag=f"tt{jm}", name=f"tt{jm}") for jm in range(JT)
                ]
                if b == 0:
                    # kt-outer across all 8 jm: each kt step needs only chunk
                    # kt (1.73us of matmuls per chunk tracks DMA arrivals);
                    # the per-jm copies stagger off the last kt round fast
                    # enough (2 engines) that matmul2 never stalls.
                    ps_l = [
                        ps_pool.tile([128, S], f32, tag="ps", name=f"ps{jm}")
                        for jm in range(JT)
                    ]
                    for kt in range(KT):
                        for jm in range(JT):
                            nc.tensor.matmul(
                                ps_l[jm][:],
                                lhsT=u0_ap(kt, jm),
                                rhs=chunk_ap(dt_x, kt),
                                start=(kt == 0),
                                stop=(kt == KT - 1),
                            )
                    for jm in range(JT):
                        tt_copy(jm, tt_t[jm], ps_l[jm])
                else:
                    for jm in range(JT):
                        ps = ps_pool.tile([128, S], f32, tag="ps", name="ps")
                        for kt in range(KT):
                            nc.tensor.matmul(
                                ps[:],
                                lhsT=u0_ap(kt, jm),
                                rhs=chunk_ap(dt_x, kt),
                                start=(kt == 0),
                                stop=(kt == KT - 1),
                            )
                        tt_copy(jm, tt_t[jm], ps)

                # matmul2: out[xt*128+p, y]  (+ bias dlin)
                for xt in range(XT):
                    dc = dcol[:, b * XT + xt : b * XT + xt + 1]
                    if b == BPC - 1 and xt == XT - 1:
                        # Final tile: 4 column groups in SEPARATE PSUM tiles
                        # (a shared tile would serialize group g+1's matmuls
                        # behind group g's copy) so bias copies + stores
                        # pipeline with the last matmuls. The slow scalar
                        # queue gets only the earliest strip.
                        ot = ot_pool.tile([128, S], f32, tag="ot", name="ot")
                        # Uneven strips: the LAST one is smallest since its
                        # copy+store latency is the drain's critical path.
                        bounds = [0, 160, 320, 448, 512]
                        for g in range(4):
                            cs = slice(bounds[g], bounds[g + 1])
                            w = bounds[g + 1] - bounds[g]
                            po = ps_pool.tile([128, S], f32, tag="ps", name=f"po{g}")
                            for jm in range(JT):
                                nc.tensor.matmul(
                                    po[:, :w],
                                    lhsT=tt_t[jm][:, xt * 128 : (xt + 1) * 128],
                                    rhs=chunk_ap(ht_x, jm, cs),
                                    start=(jm == 0),
                                    stop=(jm == JT - 1),
                                )
                            # Copies on the DVE only: the scalar ENGINE must
                            # stay free to issue its strip stores promptly.
                            nc.vector.tensor_scalar_add(ot[:, cs], po[:, :w], dc)
                            (nc.scalar if g % 2 == 0 else nc.sync).dma_start(
                                out_t[b, xt * 128 : (xt + 1) * 128, cs], ot[:, cs]
                            )
                    else:
                        po = ps_pool.tile([128, S], f32, tag="ps", name="po")
                        for jm in range(JT):
                            nc.tensor.matmul(
                                po[:],
                                lhsT=tt_t[jm][:, xt * 128 : (xt + 1) * 128],
                                rhs=chunk_ap(ht_x, jm),
                                start=(jm == 0),
                                stop=(jm == JT - 1),
                            )
                        ot = ot_pool.tile([128, S], f32, tag="ot", name="ot")
                        nc.vector.tensor_scalar_add(ot[:], po[:], dc)
                        # b3-xt2 goes on scalar so the sync queue is empty
                        # for the final tile's strips right behind it.
                        ring = nc.sync if xt % 2 == 0 else nc.scalar
                        if b == BPC - 1 and xt == XT - 2:
                            ring = nc.scalar
                        ring.dma_start(
                            out_t[b, xt * 128 : (xt + 1) * 128, :], ot[:]
                        )
    nc.finalize()
    return nc


def _get_nc() -> bass.Bass:
    key = "nc_f16" if _use_f16() else "nc_f32r"
    if key not in _NC_CACHE:
        _NC_CACHE[key] = _build_nc()
    return _NC_CACHE[key]


def _round_fp32r(a: np.ndarray) -> np.ndarray:
    """Round fp32 to fp32r layout: RNE to 11-bit mantissa, low 12 bits zero."""
    bits = np.ascontiguousarray(a, dtype=np.float32).view(np.uint32)
    odd = (bits >> 12) & np.uint32(1)
    out = (bits + np.uint32(0x7FF) + odd) & np.uint32(0xFFFFF000)
    return out.view(np.float32)


def kernel(D, H, U, W):
    D = np.ascontiguousarray(np.asarray(D, dtype=np.float32))
    H = np.ascontiguousarray(np.asarray(H, dtype=np.float32))
    U = np.asarray(U, dtype=np.float32)
    W = np.asarray(W, dtype=np.float32)
    d = D_DIM
    f16 = _use_f16()

    def _mm_cast(a):
        return a.astype(np.float16) if f16 else _round_fp32r(a)

    # U0 swizzled to [128, KT, d]: [p, kt, j] = U0[kt*128+p, j]
    U0 = _mm_cast(
        np.ascontiguousarray(U[:d, :].reshape(KT, 128, d).transpose(1, 0, 2))
    )
    c = (U[d, :] + W[d + 1 :]).astype(np.float32)  # [d]
    # ccol[p, jm] = c[jm*128 + p]
    ccol = np.ascontiguousarray(c.reshape(JT, 128).T)
    # dlin[b, x] = D[b, x] . W[:d] + W[d]
    dlin = (D @ W[:d] + W[d]).astype(np.float32)  # [B, S]

    in_maps = []
    for cidx in range(N_CORES):
        sl = slice(cidx * BPC, (cidx + 1) * BPC)
        # [b, p, kt, x] = X[b, x, kt*128+p]  (transpose + swizzle in one copy)
        Dt = _mm_cast(D[sl].reshape(BPC, S, KT, 128).transpose(0, 3, 2, 1))
        Ht = _mm_cast(H[sl].reshape(BPC, S, KT, 128).transpose(0, 3, 2, 1))
        # dcol[p, b*XT + xt] = dlin[b, xt*128 + p]
        dcol = np.ascontiguousarray(
            dlin[sl].reshape(BPC, XT, 128).transpose(2, 0, 1).reshape(128, BPC * XT)
        )
        in_maps.append(
            {
                "dt_in": Dt,
                "ht_in": Ht,
                "u0_in": U0,
                "ccol_in": ccol,
                "dcol_in": dcol,
            }
        )

    nc = _get_nc()
    trace = bool(int(os.environ.get("BIAFFINE_TRACE", "0")))
    kwargs = {}
    if trace:
        tdir = os.environ.get("BIAFFINE_TRACE_DIR")
        if tdir:
            os.makedirs(tdir, exist_ok=True)
            kwargs["tmpdir"] = tdir
    res = run_bass_kernel_spmd(
        nc, in_maps, core_ids=list(range(N_CORES)), trace=trace, **kwargs
    )
    if trace and res.exec_time_ns is not None:
        print(f"HW exec time: {res.exec_time_ns} ns")

    out = np.concatenate([res.results[i]["out"] for i in range(N_CORES)], axis=0)
    return out
